# revision 16
# baseline (speedup 1.0000x reference)
"""Trainium2 Bass kernel for a 2-layer GAT (graph attention network).

Strategy (8 NeuronCores, SPMD single program):
  - Nodes are partitioned contiguously across the 8 cores by destination;
    within each core the owned nodes are sorted by in-degree (descending)
    and laid out in chunks of 128 (padded-CSR slot layout [P, D_k]).
  - Every core builds the full layer-1 node table T1[pos] = h (128 bf16,
    256B rows) with one matmul per 128 nodes from host-transposed x.
  - Edge gathering uses gpsimd dma_gather (InstDMAGatherAnt): one
    instruction gathers up to 1024 rows from DRAM with SIMD-generated
    descriptors spread over all 16 DMA engines.  dma_gather indices are
    int16 (< 32768) so rows are PAIR-packed: table row j = nodes (2j, 2j+1),
    idx = pos >> 1, and a cheap DVE select with host-precomputed parity
    masks picks the right half per slot.
  - Attention logits: a_src per slot is recovered from the gathered h by a
    DVE multiply + reduce against the (replicated) a_src vector; a_dst of
    the owned destinations is computed by a tiny per-chunk matmul from
    per-core x columns.  Softmax over slots runs on DVE/ACT per head;
    rows are scaled by the unnormalized attention and summed on the tensor
    engine (identity-weight accumulating matmuls into PSUM).
  - Pad slots point at pad-position rows whose h solves
    a_src^T h = -500 per head (host injects x_pad = W1^{-T} h_pad into the
    padded x columns), so exp() underflows to exactly 0 after leaky-relu.
  - Layer 2: h2 (+ its pad injection via r1_pad) is computed per chunk,
    all-gathered across the 8 cores (bf16), and layer 2 repeats the same
    pair-gather/select/softmax/weighted-sum with 1 head.
  - Host does only integer graph partitioning and the final inverse
    permutation.
"""

import math

import numpy as np

# ---- problem constants (test code may override these before calling kernel) ----
N = 50000
E = 1600000
IN_CH = 128
HEADS = 4
MID = 32
OUT_CH = 64
NEG_SLOPE = 0.2
N_CORES = 8
P = 128

PADM = 500.0              # pad-slot a_src magnitude (post-leaky ~ -100)
SEG = 8                   # slot-columns per dma_gather (<= 1024 idx)
DEBUG_PHASE = 3           # 0: A only, 1: +gather, 2: +B, 3: full

_cache = {}


def _host_prep(x, edge_index):
    n_own = N // N_CORES
    assert N % N_CORES == 0 and N_CORES % 2 == 0
    K = math.ceil(n_own / P)
    ppc = K * P
    n_pad = ppc - n_own
    n_pos = ppc * N_CORES
    assert n_pad >= 1, "need at least one pad row per core"
    assert n_pos % 2 == 0

    src = np.asarray(edge_index[0], dtype=np.int64)
    dst = np.asarray(edge_index[1], dtype=np.int64)
    loops = np.arange(N, dtype=np.int64)
    src = np.concatenate([src, loops])
    dst = np.concatenate([dst, loops])

    deg = np.bincount(dst, minlength=N)
    core_of = np.arange(N) // n_own

    order = np.lexsort((-deg, core_of))
    pos_of_node = np.empty(N, np.int64)
    node_at_pos = np.full(n_pos, -1, np.int64)
    for c in range(N_CORES):
        nodes = order[c * n_own:(c + 1) * n_own]
        p0 = c * ppc
        pos_of_node[nodes] = p0 + np.arange(n_own)
        node_at_pos[p0:p0 + n_own] = nodes

    # CSR over dst
    eorder = np.argsort(dst, kind="stable")
    srcs_sorted = src[eorder]
    dst_sorted = dst[eorder]
    indptr = np.zeros(N + 1, np.int64)
    indptr[1:] = np.cumsum(deg)

    deg_pos = np.zeros(n_pos, np.int64)
    m = node_at_pos >= 0
    deg_pos[m] = deg[node_at_pos[m]]
    D_list = np.maximum(
        deg_pos.reshape(N_CORES, K, P).max(axis=(0, 2)), 1).astype(np.int64)
    offs = np.zeros(K + 1, np.int64)
    offs[1:] = np.cumsum(D_list)
    S = int(offs[-1])

    ranks = np.arange(len(dst_sorted)) - indptr[dst_sorted]
    pos_d = pos_of_node[dst_sorted]
    pos_s = pos_of_node[srcs_sorted]
    c_arr = pos_d // ppc
    rem = pos_d % ppc
    k_arr = rem // P
    p_arr = rem % P
    col = offs[k_arr] + ranks

    # slot -> source position; pads point at the owning core's first pad row
    idx = np.empty((N_CORES, P, S), np.int64)
    for c in range(N_CORES):
        idx[c].fill(c * ppc + n_own)
    idx[c_arr, p_arr, col] = pos_s

    # dma_gather index arrays: int16 pair-word idx, wrapped per 16 partitions
    # per instruction segment; plus bf16 parity masks [P, S] (m, 1-m).
    idxw = np.empty((N_CORES, 128, 8 * S), np.int16)
    par = (idx & 1).astype(np.float32)
    segs = []               # (k, d0, d1) instruction segments, shared layout
    for k in range(K):
        D = int(D_list[k])
        for d0 in range(0, D, SEG):
            segs.append((k, int(offs[k]) + d0, int(offs[k]) + min(d0 + SEG, D)))
    for c in range(N_CORES):
        w = (idx[c] >> 1).astype(np.int16)          # [P, S]
        for (_, s0, s1) in segs:
            flat = w[:, s0:s1].T.reshape(-1)        # j = (d-d0)*128 + p
            blk = flat.reshape(-1, 16).T            # [16, nidx/16]
            idxw[c][:, 8 * s0:8 * s1] = np.tile(blk, (8, 1))

    xT = np.zeros((IN_CH, n_pos), np.float32)
    xT[:, m] = np.asarray(x, np.float32)[node_at_pos[m]].T

    return dict(n_own=n_own, K=K, ppc=ppc, n_pad=n_pad, n_pos=n_pos,
                D_list=tuple(int(v) for v in D_list),
                offs=tuple(int(v) for v in offs), S=S,
                idxw=idxw, par=par, xT=xT, node_at_pos=node_at_pos)


def _feat_mat(a, heads, mid):
    """[heads, mid] attention vector -> block-diagonal [heads*mid, heads]."""
    a = np.asarray(a, np.float32)
    out = np.zeros((heads * mid, heads), np.float32)
    for h in range(heads):
        out[h * mid:(h + 1) * mid, h] = a[h]
    return out


def _finish(nc):
    nc.compile()
    return nc


def _build_program(K, D_list, offs, S, n_pos, ppc, n_own, n_pad):
    import concourse.bass as bass
    import concourse.mybir as mybir
    import concourse.tile as tile
    from concourse.bacc import Bacc
    from concourse.masks import make_identity

    f32 = mybir.dt.float32
    bf16 = mybir.dt.bfloat16
    i16 = mybir.dt.int16
    Alu = mybir.AluOpType
    Act = mybir.ActivationFunctionType

    HM = HEADS * MID              # 128
    C_W1 = 0                      # [P, 128] W1 (rows = input channel)
    C_A1D = C_W1 + HM             # [P, 4]   A1d feature-space
    C_W2A = C_A1D + HEADS         # [P, 65]  [W2 | W2 @ a_dst2]
    C_A1S = C_W2A + OUT_CH + 1    # [P, 128] a_src1 flat, replicated rows
    C_A2S = C_A1S + HM            # [P, 64]  a_src2 replicated rows
    C_B1 = C_A2S + OUT_CH         # [P, 128]
    C_B2 = C_B1 + HM              # [P, 64]
    C_M01 = C_B2 + OUT_CH         # [P, 1]   1 on real rows of last chunk
    C_PADT = C_M01 + 1            # [P, 128] r1_pad on pad rows, else 0
    C_END = C_PADT + HM

    nc = Bacc("TRN2", num_swdge_queues=4)
    xT = nc.declare_dram_parameter("xT", [IN_CH, n_pos], f32, isOutput=False)
    xTo = nc.declare_dram_parameter("xTown", [IN_CH, ppc], f32, isOutput=False)
    constP = nc.declare_dram_parameter("consts", [P, C_END], f32,
                                       isOutput=False)
    idxP = nc.declare_dram_parameter("idxw", [P, 8 * S], i16, isOutput=False)
    parP = nc.declare_dram_parameter("pmask", [P, 2 * S], bf16, isOutput=False)
    outP = nc.declare_dram_parameter("out", [ppc, OUT_CH], f32, isOutput=True)

    T1 = nc.dram_tensor("T1", [n_pos, HM], bf16)
    T2s = nc.dram_tensor("T2s", [ppc, OUT_CH], bf16)
    T2 = nc.dram_tensor("T2", [n_pos, OUT_CH], bf16, addr_space="Shared")
    T1p = T1[0:n_pos, :].rearrange("(a b) f -> a (b f)", b=2)   # [n_pos/2, 256]
    T2p = T2[0:n_pos, :].rearrange("(a b) f -> a (b f)", b=2)   # [n_pos/2, 128]

    n_tiles = n_pos // P

    with tile.TileContext(nc) as tc:
        with tc.tile_pool(name="const", bufs=1) as cpool:
            consts = cpool.tile([P, C_END], f32)
            nc.sync.dma_start(out=consts[:], in_=constP[:, :])
            w1_t = consts[:, C_W1:C_W1 + HM]
            a1d_t = consts[:, C_A1D:C_A1D + HEADS]
            w2a_t = consts[:, C_W2A:C_W2A + OUT_CH + 1]
            a1s_t = consts[:, C_A1S:C_A1S + HM]
            a2s_t = consts[:, C_A2S:C_A2S + OUT_CH]
            b1r_t = consts[:, C_B1:C_B1 + HM]
            b2r_t = consts[:, C_B2:C_B2 + OUT_CH]
            m01_t = consts[:, C_M01:C_M01 + 1]
            padt_t = consts[:, C_PADT:C_PADT + HM]

            idx_t = cpool.tile([P, 8 * S], i16)
            nc.sync.dma_start(out=idx_t[:], in_=idxP[:, :])
            par_t = cpool.tile([P, 2 * S], bf16)
            nc.sync.dma_start(out=par_t[:], in_=parP[:, :])

            ident_b = cpool.tile([P, P], bf16)
            make_identity(nc, ident_b[:])
            padt_b = cpool.tile([P, HM], bf16)
            nc.vector.tensor_copy(padt_b[:], padt_t)
            if DEBUG_PHASE >= 2:
                t2stage = cpool.tile([P, K * OUT_CH], bf16)
                adst2O = cpool.tile([P, K], f32)
            adstO = cpool.tile([P, HEADS * K], f32)

            # ---------------- phase A: build T1 rows (h) for all positions --
            NSPLIT = max(1, min(8, n_tiles // 8))
            bounds = [n_tiles * i // NSPLIT for i in range(NSPLIT + 1)]
            with tc.tile_pool(name="pa_x", bufs=3) as xpool, \
                 tc.tile_pool(name="pa_st", bufs=2) as stpool, \
                 tc.tile_pool(name="pa_ps", bufs=2, space="PSUM") as pspool:
                GRP = 8
                for s_ in range(NSPLIT):
                    lo, hi = bounds[s_], bounds[s_ + 1]
                    stg = stpool.tile([P, (hi - lo) * HM], bf16, tag="stg")
                    for t0 in range(lo, hi, GRP):
                        g = min(GRP, hi - t0)
                        xt = xpool.tile([IN_CH, g * P], f32, tag="xt")
                        nc.sync.dma_start(out=xt[:],
                                          in_=xT[:, t0 * P:(t0 + g) * P])
                        for j in range(g):
                            ps = pspool.tile([P, HM], f32, tag="ps")
                            nc.tensor.matmul(ps[:],
                                             lhsT=xt[:, j * P:(j + 1) * P],
                                             rhs=w1_t, start=True, stop=True)
                            t = t0 + j
                            nc.scalar.copy(
                                stg[:, (t - lo) * HM:(t - lo + 1) * HM], ps[:])
                    dview = T1[lo * P:hi * P, :].rearrange(
                        "(t p) f -> p t f", p=P)
                    nc.sync.dma_start(
                        out=dview,
                        in_=stg[:].rearrange("p (t f) -> p t f", f=HM))

                # phase A2: a_dst of owned nodes, per chunk
                xto = xpool.tile([IN_CH, ppc], f32, tag="xto")
                nc.sync.dma_start(out=xto[:], in_=xTo[:, :])
                for k in range(K):
                    ps2 = pspool.tile([P, HEADS], f32, tag="ps2")
                    nc.tensor.matmul(ps2[:],
                                     lhsT=xto[:, k * P:(k + 1) * P],
                                     rhs=a1d_t, start=True, stop=True)
                    nc.vector.tensor_copy(
                        adstO[:, HEADS * k:HEADS * (k + 1)], ps2[:])

            # ---------------- phase B: layer-1 aggregation ------------------
            with tc.tile_pool(name="pb_g", bufs=2) as gpool, \
                 tc.tile_pool(name="pb_sm", bufs=3) as smpool, \
                 tc.tile_pool(name="pb_ps", bufs=2, space="PSUM") as psB, \
                 tc.tile_pool(name="pb_pst", bufs=2, space="PSUM") as psT, \
                 tc.tile_pool(name="pb_psu", bufs=2, space="PSUM") as psU:
                qrot = 0
                for k in range(K):
                    D = D_list[k]
                    co = offs[k]
                    G = gpool.tile([P, D, 2 * HM], bf16, tag="G")
                    if DEBUG_PHASE < 1:
                        nc.vector.memset(G[:], 0)
                    for d0 in range(0, D, SEG) if DEBUG_PHASE >= 1 else []:
                        d1 = min(d0 + SEG, D)
                        nidx = 128 * (d1 - d0)
                        nc.gpsimd.dma_gather(
                            G[:, d0:d1, :], T1p,
                            idx_t[:, 8 * (co + d0):8 * (co + d1)],
                            nidx, nidx, 2 * HM, queue_num=qrot % 4)
                        qrot += 1
                    if DEBUG_PHASE < 2:
                        dbg = smpool.tile([P, OUT_CH], f32, tag="dbg")
                        nc.vector.tensor_copy(dbg[:], G[:, 0, 0:OUT_CH])
                        nc.sync.dma_start(out=outP[k * P:(k + 1) * P, :],
                                          in_=dbg[:])
                        continue
                    ev = G[:, :, 0:HM]
                    od = G[:, :, HM:2 * HM]
                    mi_b = par_t[:, S + co:S + co + D].unsqueeze(2) \
                        .to_broadcast([P, D, HM])
                    m_b = par_t[:, co:co + D].unsqueeze(2) \
                        .to_broadcast([P, D, HM])
                    nc.vector.tensor_tensor(out=ev, in0=ev, in1=mi_b,
                                            op=Alu.mult)
                    nc.vector.tensor_tensor(out=od, in0=od, in1=m_b,
                                            op=Alu.mult)
                    nc.vector.tensor_tensor(out=ev, in0=ev, in1=od,
                                            op=Alu.add)

                    # logits: a_src via mult+reduce, + a_dst, leaky-relu
                    logits = smpool.tile([P, HEADS * D], f32, tag="logits")
                    tmpm = smpool.tile([P, D, MID], f32, tag="tmpm")
                    for h in range(HEADS):
                        a_b = a1s_t[:, h * MID:(h + 1) * MID].unsqueeze(1) \
                            .to_broadcast([P, D, MID])
                        nc.vector.tensor_tensor(
                            out=tmpm[:], in0=ev[:, :, h * MID:(h + 1) * MID],
                            in1=a_b, op=Alu.mult)
                        lh = logits[:, h * D:(h + 1) * D]
                        nc.vector.tensor_reduce(
                            lh, tmpm[:], axis=mybir.AxisListType.X,
                            op=Alu.add)
                        nc.vector.tensor_scalar_add(
                            lh, lh, adstO[:, HEADS * k + h:HEADS * k + h + 1])
                        nc.vector.scalar_tensor_tensor(
                            lh, lh, NEG_SLOPE, lh, op0=Alu.mult, op1=Alu.max)
                    negmax = smpool.tile([P, HEADS], f32, tag="negmax")
                    for h in range(HEADS):
                        nc.vector.reduce_max(
                            negmax[:, h:h + 1], logits[:, h * D:(h + 1) * D],
                            axis=mybir.AxisListType.X, negate=True)
                    e_t = smpool.tile([P, HEADS * D], f32, tag="e")
                    s_t = smpool.tile([P, HEADS], f32, tag="s")
                    for h in range(HEADS):
                        nc.scalar.activation(
                            e_t[:, h * D:(h + 1) * D],
                            logits[:, h * D:(h + 1) * D],
                            Act.Exp, bias=negmax[:, h:h + 1],
                            accum_out=s_t[:, h:h + 1])
                    rcp = smpool.tile([P, HEADS], f32, tag="rcp")
                    nc.vector.reciprocal(rcp[:], s_t[:])

                    # scale gathered h in place by unnormalized attention
                    hview = ev.rearrange("p d (h c) -> p d h c", c=MID)
                    e_b = e_t[:].rearrange("p (h d) -> p d h", d=D) \
                        .unsqueeze(3).to_broadcast([P, D, HEADS, MID])
                    nc.vector.tensor_tensor(out=hview, in0=hview, in1=e_b,
                                            op=Alu.mult)

                    ps = psB.tile([P, HM], f32, tag="acc")
                    for d in range(D):
                        nc.tensor.matmul(ps[:], lhsT=ident_b[:],
                                         rhs=ev[:, d, :],
                                         start=(d == 0), stop=(d == D - 1))

                    tmp = smpool.tile([P, HM], f32, tag="tmp")
                    rcp_b = rcp[:].unsqueeze(2).to_broadcast([P, HEADS, MID])
                    nc.vector.tensor_tensor(
                        out=tmp[:].rearrange("p (h c) -> p h c", c=MID),
                        in0=ps[:].rearrange("p (h c) -> p h c", c=MID),
                        in1=rcp_b, op=Alu.mult)
                    nc.vector.tensor_add(tmp[:], tmp[:], b1r_t)
                    r1 = smpool.tile([P, HM], bf16, tag="r1")
                    nc.scalar.activation(r1[:], tmp[:], Act.Relu)
                    if k == K - 1 and n_pad:
                        # replace pad rows with r1_pad (a_src2 -> -PADM)
                        nc.vector.tensor_scalar_mul(r1[:], r1[:],
                                                    m01_t[:, 0:1])
                        nc.vector.tensor_add(r1[:], r1[:], padt_b[:])

                    tps = psT.tile([P, P], bf16, tag="tps")
                    nc.tensor.transpose(tps[:], r1[:], ident_b[:])
                    r1T = smpool.tile([P, P], f32, tag="r1T")
                    nc.vector.tensor_copy(r1T[:], tps[:])
                    t2ps = psU.tile([P, OUT_CH + 1], f32, tag="t2ps")
                    nc.tensor.matmul(t2ps[:], lhsT=r1T[:], rhs=w2a_t,
                                     start=True, stop=True)
                    nc.scalar.copy(
                        t2stage[:, k * OUT_CH:(k + 1) * OUT_CH],
                        t2ps[:, 0:OUT_CH])
                    nc.vector.tensor_copy(adst2O[:, k:k + 1],
                                          t2ps[:, OUT_CH:OUT_CH + 1])

            if DEBUG_PHASE >= 2:
                nc.sync.dma_start(
                    out=T2s[:, :].rearrange("(k p) f -> p k f", p=P),
                    in_=t2stage[:].rearrange("p (k f) -> p k f", f=OUT_CH))

                nc.gpsimd.collective_compute(
                    "AllGather",
                    mybir.AluOpType.bypass,
                    replica_groups=[list(range(N_CORES))],
                    ins=[T2s[:, :]],
                    outs=[T2[:, :]],
                )

            # ---------------- phase C: layer-2 aggregation ------------------
            if DEBUG_PHASE >= 3:
              with tc.tile_pool(name="pc_g", bufs=2) as g2pool, \
                 tc.tile_pool(name="pc_sm", bufs=3) as sm2pool, \
                 tc.tile_pool(name="pc_ps", bufs=2, space="PSUM") as psC:
                qrot = 0
                for k in range(K):
                    D = D_list[k]
                    co = offs[k]
                    G2 = g2pool.tile([P, D, 2 * OUT_CH], bf16, tag="G2")
                    for d0 in range(0, D, SEG):
                        d1 = min(d0 + SEG, D)
                        nidx = 128 * (d1 - d0)
                        nc.gpsimd.dma_gather(
                            G2[:, d0:d1, :], T2p,
                            idx_t[:, 8 * (co + d0):8 * (co + d1)],
                            nidx, nidx, 2 * OUT_CH, queue_num=qrot % 4)
                        qrot += 1
                    ev2 = G2[:, :, 0:OUT_CH]
                    od2 = G2[:, :, OUT_CH:2 * OUT_CH]
                    mi_b = par_t[:, S + co:S + co + D].unsqueeze(2) \
                        .to_broadcast([P, D, OUT_CH])
                    m_b = par_t[:, co:co + D].unsqueeze(2) \
                        .to_broadcast([P, D, OUT_CH])
                    nc.vector.tensor_tensor(out=ev2, in0=ev2, in1=mi_b,
                                            op=Alu.mult)
                    nc.vector.tensor_tensor(out=od2, in0=od2, in1=m_b,
                                            op=Alu.mult)
                    nc.vector.tensor_tensor(out=ev2, in0=ev2, in1=od2,
                                            op=Alu.add)

                    logits2 = sm2pool.tile([P, D], f32, tag="logits2")
                    tmp2 = sm2pool.tile([P, D, OUT_CH], f32, tag="tmp2")
                    a2_b = a2s_t[:, :].unsqueeze(1) \
                        .to_broadcast([P, D, OUT_CH])
                    nc.vector.tensor_tensor(out=tmp2[:], in0=ev2, in1=a2_b,
                                            op=Alu.mult)
                    nc.vector.tensor_reduce(
                        logits2[:], tmp2[:], axis=mybir.AxisListType.X,
                        op=Alu.add)
                    nc.vector.tensor_scalar_add(logits2[:], logits2[:],
                                                adst2O[:, k:k + 1])
                    nc.vector.scalar_tensor_tensor(
                        logits2[:], logits2[:], NEG_SLOPE, logits2[:],
                        op0=Alu.mult, op1=Alu.max)
                    negmax2 = sm2pool.tile([P, 1], f32, tag="negmax2")
                    nc.vector.reduce_max(negmax2[:], logits2[:],
                                         axis=mybir.AxisListType.X,
                                         negate=True)
                    e2 = sm2pool.tile([P, D], f32, tag="e2")
                    s2 = sm2pool.tile([P, 1], f32, tag="s2")
                    nc.scalar.activation(e2[:], logits2[:], Act.Exp,
                                         bias=negmax2[:, 0:1],
                                         accum_out=s2[:, 0:1])
                    rcp2 = sm2pool.tile([P, 1], f32, tag="rcp2")
                    nc.vector.reciprocal(rcp2[:], s2[:])

                    e2_b = e2[:].unsqueeze(2).to_broadcast([P, D, OUT_CH])
                    nc.vector.tensor_tensor(out=ev2, in0=ev2, in1=e2_b,
                                            op=Alu.mult)

                    ps2 = psC.tile([P, OUT_CH], f32, tag="acc2")
                    for d in range(D):
                        nc.tensor.matmul(ps2[:], lhsT=ident_b[:],
                                         rhs=ev2[:, d, :],
                                         start=(d == 0), stop=(d == D - 1))

                    outt = sm2pool.tile([P, OUT_CH], f32, tag="outt")
                    nc.scalar.activation(outt[:], ps2[:], Act.Identity,
                                         scale=rcp2[:, 0:1])
                    nc.vector.tensor_add(outt[:], outt[:], b2r_t)
                    nc.sync.dma_start(out=outP[k * P:(k + 1) * P, :],
                                      in_=outt[:])

    return _finish(nc)


def _make_runner(nc, n_cores):
    import jax
    from jax.sharding import Mesh, PartitionSpec
    from jax.experimental.shard_map import shard_map
    from concourse import bass2jax
    import concourse.mybir as mybir

    bass2jax.install_neuronx_cc_hook()
    partition_name = (nc.partition_id_tensor.name
                      if nc.partition_id_tensor else None)
    in_names = []
    out_names = []
    out_avals = []
    zero_outs = []
    for alloc in nc.m.functions[0].allocations:
        if not isinstance(alloc, mybir.MemoryLocationSet):
            continue
        name = alloc.memorylocations[0].name
        if alloc.kind == "ExternalInput":
            if name != partition_name:
                in_names.append(name)
        elif alloc.kind == "ExternalOutput":
            shape = tuple(alloc.tensor_shape)
            dtype = mybir.dt.np(alloc.dtype)
            out_names.append(name)
            out_avals.append(jax.core.ShapedArray(shape, dtype))
            zero_outs.append(np.zeros(shape, dtype))
    n_params = len(in_names)
    all_names = list(in_names) + out_names
    if partition_name is not None:
        all_names.append(partition_name)

    def _body(*args):
        operands = list(args)
        if partition_name is not None:
            operands.append(bass2jax.partition_id_tensor())
        outs = bass2jax._bass_exec_p.bind(
            *operands,
            out_avals=tuple(out_avals),
            in_names=tuple(all_names),
            out_names=tuple(out_names),
            lowering_input_output_aliases=(),
            sim_require_finite=True,
            sim_require_nnan=True,
            nc=nc,
        )
        return tuple(outs)

    devices = jax.devices()[:n_cores]
    mesh = Mesh(np.asarray(devices), ("core",))
    nio = n_params + len(out_names)
    sharded = jax.jit(
        shard_map(_body, mesh=mesh, in_specs=(PartitionSpec("core"),) * nio,
                  out_specs=(PartitionSpec("core"),) * len(out_names),
                  check_rep=False),
        keep_unused=True,
    )
    return dict(fn=sharded, in_names=in_names, out_names=out_names,
                zero_outs=zero_outs, mesh=mesh, n_cores=n_cores)


def _execute(runner, in_maps):
    n_cores = runner["n_cores"]
    concat_in = [
        np.concatenate([np.asarray(in_maps[c][name])
                        for c in range(n_cores)], axis=0)
        for name in runner["in_names"]
    ]
    concat_zeros = [
        np.zeros((n_cores * z.shape[0], *z.shape[1:]), z.dtype)
        for z in runner["zero_outs"]
    ]
    out_arrs = runner["fn"](*concat_in, *concat_zeros)
    out_arrs = [np.asarray(a) for a in out_arrs]
    res = []
    for c in range(n_cores):
        m = {}
        for i, name in enumerate(runner["out_names"]):
            a = out_arrs[i]
            s0 = a.shape[0] // n_cores
            m[name] = a[c * s0:(c + 1) * s0]
        res.append(m)
    return res


def _time_exec(runner, in_maps, iters=5):
    """Steady-state wall-clock of the compiled NEFF execution (device-resident
    inputs, no host transfers in the loop)."""
    import time as _time

    import jax
    from jax.sharding import NamedSharding, PartitionSpec

    n_cores = runner["n_cores"]
    sh = NamedSharding(runner["mesh"], PartitionSpec("core"))
    concat_in = [
        np.concatenate([np.asarray(in_maps[c][name])
                        for c in range(n_cores)], axis=0)
        for name in runner["in_names"]
    ]
    concat_zeros = [
        np.zeros((n_cores * z.shape[0], *z.shape[1:]), z.dtype)
        for z in runner["zero_outs"]
    ]
    dev_in = [jax.device_put(a, sh) for a in concat_in]
    dev_z = [jax.device_put(a, sh) for a in concat_zeros]
    times = []
    for _ in range(iters):
        t0 = _time.perf_counter()
        outs = runner["fn"](*dev_in, *dev_z)
        for o in outs:
            o.block_until_ready()
        times.append(_time.perf_counter() - t0)
    return min(times), times


def _get_compiled(inputs):
    x = np.asarray(inputs["x"], np.float32)
    prep = _host_prep(x, np.asarray(inputs["edge_index"]))
    key = (prep["K"], prep["D_list"], prep["n_pos"], prep["ppc"],
           prep["n_own"], prep["n_pad"])
    if key not in _cache:
        nc = _build_program(prep["K"], prep["D_list"], prep["offs"],
                            prep["S"], prep["n_pos"], prep["ppc"],
                            prep["n_own"], prep["n_pad"])
        _cache[key] = _make_runner(nc, N_CORES)
    runner = _cache[key]

    import ml_dtypes

    W1 = np.asarray(inputs["W1"], np.float32)
    W2 = np.asarray(inputs["W2"], np.float32)
    a_src1 = np.asarray(inputs["a_src1"], np.float32)
    a_dst1 = np.asarray(inputs["a_dst1"], np.float32)
    a_src2 = np.asarray(inputs["a_src2"], np.float32).reshape(-1)
    a_dst2 = np.asarray(inputs["a_dst2"], np.float32).reshape(-1)
    HM = HEADS * MID

    A1s = _feat_mat(a_src1, HEADS, MID)          # [128, 4]
    A1d = _feat_mat(a_dst1, HEADS, MID)
    # pad h: A1s^T h = -PADM per head (least-norm), x_pad = W1^{-T} h_pad
    h_pad = A1s @ np.linalg.solve(A1s.T @ A1s,
                                  np.full(HEADS, -PADM, np.float32))
    x_pad = np.linalg.solve(W1.T, h_pad).astype(np.float32)
    # layer-2 pad: a2s^T (W2^T r1_pad) = -PADM
    h2_pad = (-PADM / (a_src2 @ a_src2)) * a_src2
    r1_pad = np.linalg.lstsq(W2.T, h2_pad, rcond=None)[0].astype(np.float32)

    # inject pad columns into xT (identical pad content on every core)
    n_own, ppc, n_pad = prep["n_own"], prep["ppc"], prep["n_pad"]
    xT = prep["xT"]
    for c in range(N_CORES):
        xT[:, c * ppc + n_own:(c + 1) * ppc] = x_pad[:, None]

    C_END = HM + HEADS + (OUT_CH + 1) + HM + OUT_CH + HM + OUT_CH + 1 + HM
    consts = np.zeros((P, C_END), np.float32)
    o = 0
    consts[:, o:o + HM] = W1; o += HM
    consts[:, o:o + HEADS] = W1 @ A1d; o += HEADS
    consts[:, o:o + OUT_CH] = W2
    consts[:, o + OUT_CH:o + OUT_CH + 1] = (W2 @ a_dst2)[:, None]
    o += OUT_CH + 1
    consts[:, o:o + HM] = np.broadcast_to(a_src1.reshape(-1)[None, :],
                                          (P, HM)); o += HM
    consts[:, o:o + OUT_CH] = np.broadcast_to(a_src2[None, :], (P, OUT_CH))
    o += OUT_CH
    consts[:, o:o + HM] = np.broadcast_to(
        np.asarray(inputs["b1"], np.float32)[None, :], (P, HM)); o += HM
    consts[:, o:o + OUT_CH] = np.broadcast_to(
        np.asarray(inputs["b2"], np.float32)[None, :], (P, OUT_CH))
    o += OUT_CH
    r = n_own % P
    m01 = np.ones(P, np.float32)
    padt = np.zeros((P, HM), np.float32)
    if n_pad:
        m01[r:] = 0.0
        padt[r:, :] = r1_pad[None, :]
    consts[:, o:o + 1] = m01[:, None]; o += 1
    consts[:, o:o + HM] = padt; o += HM

    S = prep["S"]
    in_maps = []
    for c in range(N_CORES):
        pm = np.empty((P, 2 * S), np.float32)
        pm[:, 0:S] = prep["par"][c]
        pm[:, S:2 * S] = 1.0 - prep["par"][c]
        in_maps.append({
            "xT": xT,
            "xTown": np.ascontiguousarray(xT[:, c * ppc:(c + 1) * ppc]),
            "consts": consts,
            "idxw": prep["idxw"][c],
            "pmask": pm.astype(ml_dtypes.bfloat16),
        })
    return runner, in_maps, prep


def _run(inputs):
    runner, in_maps, prep = _get_compiled(inputs)
    import time as _time
    last_exc = None
    for attempt in range(3):
        try:
            results = _execute(runner, in_maps)
            break
        except Exception as exc:
            last_exc = exc
            _time.sleep(2.0 + 4.0 * attempt)
    else:
        raise last_exc
    out = np.empty((N, OUT_CH), np.float32)
    n_own, ppc = prep["n_own"], prep["ppc"]
    for c in range(N_CORES):
        o = np.asarray(results[c]["out"])
        nodes = prep["node_at_pos"][c * ppc:c * ppc + n_own]
        out[nodes] = o[:n_own]
    return out


def kernel(**inputs):
    return _run(inputs)


# revision 29
# speedup vs baseline: 1.0583x; 1.0583x over previous
"""Trainium2 Bass kernel for a 2-layer GAT (graph attention network).

Strategy (8 NeuronCores, SPMD single program):
  - Nodes are partitioned contiguously across the 8 cores by destination;
    within each core the owned nodes are sorted by in-degree (descending)
    and laid out in chunks of 128 (padded-CSR slot layout [P, D_k]).
  - Every core builds the full layer-1 node table T1[pos] = h (128 bf16,
    256B rows) with one matmul per 128 nodes from host-transposed x.
  - Edge gathering uses gpsimd dma_gather (InstDMAGatherAnt): one
    instruction gathers up to 1024 rows from DRAM with SIMD-generated
    descriptors spread over all 16 DMA engines.  dma_gather indices are
    int16 (< 32768) so rows are PAIR-packed: table row j = nodes (2j, 2j+1),
    idx = pos >> 1, and a cheap DVE select with host-precomputed parity
    masks picks the right half per slot.
  - Attention logits: a_src per slot is recovered from the gathered h by a
    DVE multiply + reduce against the (replicated) a_src vector; a_dst of
    the owned destinations is computed by a tiny per-chunk matmul from
    per-core x columns.  Softmax over slots runs on DVE/ACT per head;
    rows are scaled by the unnormalized attention and summed on the tensor
    engine (identity-weight accumulating matmuls into PSUM).
  - Pad slots point at pad-position rows whose h solves
    a_src^T h = -500 per head (host injects x_pad = W1^{-T} h_pad into the
    padded x columns), so exp() underflows to exactly 0 after leaky-relu.
  - Layer 2: h2 (+ its pad injection via r1_pad) is computed per chunk,
    all-gathered across the 8 cores (bf16), and layer 2 repeats the same
    pair-gather/select/softmax/weighted-sum with 1 head.
  - Host does only integer graph partitioning and the final inverse
    permutation.
"""

import math

import numpy as np

# ---- problem constants (test code may override these before calling kernel) ----
N = 50000
E = 1600000
IN_CH = 128
HEADS = 4
MID = 32
OUT_CH = 64
NEG_SLOPE = 0.2
N_CORES = 8
P = 128

PADM = 500.0              # pad-slot a_src magnitude (post-leaky ~ -100)
SEG = 8                   # slot-columns per dma_gather (<= 1024 idx)
DEBUG_PHASE = 3           # 0: A only, 1: +gather, 2: +B, 3: full

_cache = {}


def _host_prep(x, edge_index):
    n_own = N // N_CORES
    assert N % N_CORES == 0 and N_CORES % 2 == 0
    K = math.ceil(n_own / P)
    ppc = K * P
    n_pad = ppc - n_own
    n_pos = ppc * N_CORES
    assert n_pad >= 1, "need at least one pad row per core"
    assert n_pos % 2 == 0

    src = np.asarray(edge_index[0], dtype=np.int64)
    dst = np.asarray(edge_index[1], dtype=np.int64)
    loops = np.arange(N, dtype=np.int64)
    src = np.concatenate([src, loops])
    dst = np.concatenate([dst, loops])

    deg = np.bincount(dst, minlength=N)
    core_of = np.arange(N) // n_own

    order = np.lexsort((-deg, core_of))
    pos_of_node = np.empty(N, np.int64)
    node_at_pos = np.full(n_pos, -1, np.int64)
    for c in range(N_CORES):
        nodes = order[c * n_own:(c + 1) * n_own]
        p0 = c * ppc
        pos_of_node[nodes] = p0 + np.arange(n_own)
        node_at_pos[p0:p0 + n_own] = nodes

    # CSR over dst
    eorder = np.argsort(dst, kind="stable")
    srcs_sorted = src[eorder]
    dst_sorted = dst[eorder]
    indptr = np.zeros(N + 1, np.int64)
    indptr[1:] = np.cumsum(deg)

    deg_pos = np.zeros(n_pos, np.int64)
    m = node_at_pos >= 0
    deg_pos[m] = deg[node_at_pos[m]]
    D_list = np.maximum(
        deg_pos.reshape(N_CORES, K, P).max(axis=(0, 2)), 1).astype(np.int64)
    offs = np.zeros(K + 1, np.int64)
    offs[1:] = np.cumsum(D_list)
    S = int(offs[-1])

    ranks = np.arange(len(dst_sorted)) - indptr[dst_sorted]
    pos_d = pos_of_node[dst_sorted]
    pos_s = pos_of_node[srcs_sorted]
    c_arr = pos_d // ppc
    rem = pos_d % ppc
    k_arr = rem // P
    p_arr = rem % P
    col = offs[k_arr] + ranks

    # slot -> source position; pads point at the owning core's first pad row
    idx = np.empty((N_CORES, P, S), np.int64)
    for c in range(N_CORES):
        idx[c].fill(c * ppc + n_own)
    idx[c_arr, p_arr, col] = pos_s

    # dma_gather index arrays: int16 pair-word idx, wrapped per 16 partitions
    # per instruction segment; plus bf16 parity masks [P, S] (m, 1-m).
    idxw = np.empty((N_CORES, 128, 8 * S), np.int16)
    par = (idx & 1).astype(np.float32)
    segs = []               # (k, d0, d1) instruction segments, shared layout
    for k in range(K):
        D = int(D_list[k])
        for d0 in range(0, D, SEG):
            segs.append((k, int(offs[k]) + d0, int(offs[k]) + min(d0 + SEG, D)))
    for c in range(N_CORES):
        w = (idx[c] >> 1).astype(np.int16)          # [P, S]
        for (_, s0, s1) in segs:
            flat = w[:, s0:s1].T.reshape(-1)        # j = (d-d0)*128 + p
            blk = flat.reshape(-1, 16).T            # [16, nidx/16]
            idxw[c][:, 8 * s0:8 * s1] = np.tile(blk, (8, 1))

    xT = np.zeros((IN_CH, n_pos), np.float32)
    xT[:, m] = np.asarray(x, np.float32)[node_at_pos[m]].T

    return dict(n_own=n_own, K=K, ppc=ppc, n_pad=n_pad, n_pos=n_pos,
                D_list=tuple(int(v) for v in D_list),
                offs=tuple(int(v) for v in offs), S=S,
                idxw=idxw, par=par, xT=xT, node_at_pos=node_at_pos)


def _feat_mat(a, heads, mid):
    """[heads, mid] attention vector -> block-diagonal [heads*mid, heads]."""
    a = np.asarray(a, np.float32)
    out = np.zeros((heads * mid, heads), np.float32)
    for h in range(heads):
        out[h * mid:(h + 1) * mid, h] = a[h]
    return out


def _finish(nc):
    nc.compile()
    return nc


def _build_program(K, D_list, offs, S, n_pos, ppc, n_own, n_pad):
    import concourse.bass as bass
    import concourse.mybir as mybir
    import concourse.tile as tile
    from concourse.bacc import Bacc
    from concourse.masks import make_identity

    f32 = mybir.dt.float32
    bf16 = mybir.dt.bfloat16
    i16 = mybir.dt.int16
    Alu = mybir.AluOpType
    Act = mybir.ActivationFunctionType

    HM = HEADS * MID              # 128
    C_W1 = 0                      # [P, 128] W1 (rows = input channel)
    C_A1D = C_W1 + HM             # [P, 4]   A1d feature-space
    C_W2A = C_A1D + HEADS         # [P, 65]  [W2 | W2 @ a_dst2]
    C_A1S = C_W2A + OUT_CH + 1    # [P, 128] a_src1 flat, replicated rows
    C_A2S = C_A1S + HM            # [P, 64]  a_src2 replicated rows
    C_B1 = C_A2S + OUT_CH         # [P, 128]
    C_B2 = C_B1 + HM              # [P, 64]
    C_M01 = C_B2 + OUT_CH         # [P, 1]   1 on real rows, 0 on pad rows
    C_HPAD = C_M01 + 1            # [P, 128] h_pad on pad rows, else 0
    C_H2PAD = C_HPAD + HM         # [P, 64]  h2_pad on pad rows, else 0
    C_END = C_H2PAD + OUT_CH

    nc = Bacc("TRN2", num_swdge_queues=4)
    xT = nc.declare_dram_parameter("xT", [IN_CH, n_pos], f32, isOutput=False)
    xTo = nc.declare_dram_parameter("xTown", [IN_CH, ppc], f32, isOutput=False)
    constP = nc.declare_dram_parameter("consts", [P, C_END], f32,
                                       isOutput=False)
    idxP = nc.declare_dram_parameter("idxw", [P, 8 * S], i16, isOutput=False)
    parP = nc.declare_dram_parameter("pmask", [P, 2 * S], bf16, isOutput=False)
    outP = nc.declare_dram_parameter("out", [ppc, OUT_CH], f32, isOutput=True)

    T1 = nc.dram_tensor("T1", [n_pos, HM], bf16)
    T2s = nc.dram_tensor("T2s", [ppc, OUT_CH], bf16)
    T2 = nc.dram_tensor("T2", [n_pos, OUT_CH], bf16, addr_space="Shared")
    T1p = T1[0:n_pos, :].rearrange("(a b) f -> a (b f)", b=2)   # [n_pos/2, 256]
    T2p = T2[0:n_pos, :].rearrange("(a b) f -> a (b f)", b=2)   # [n_pos/2, 128]

    n_tiles = n_pos // P

    with tile.TileContext(nc) as tc:
        with tc.tile_pool(name="const", bufs=1) as cpool:
            consts = cpool.tile([P, C_END], f32)
            nc.sync.dma_start(out=consts[:], in_=constP[:, :])
            w1_t = consts[:, C_W1:C_W1 + HM]
            a1d_t = consts[:, C_A1D:C_A1D + HEADS]
            w2a_t = consts[:, C_W2A:C_W2A + OUT_CH + 1]
            a1s_t = consts[:, C_A1S:C_A1S + HM]
            a2s_t = consts[:, C_A2S:C_A2S + OUT_CH]
            b1r_t = consts[:, C_B1:C_B1 + HM]
            b2r_t = consts[:, C_B2:C_B2 + OUT_CH]
            m01_t = consts[:, C_M01:C_M01 + 1]
            hpad_t = consts[:, C_HPAD:C_HPAD + HM]
            h2pad_t = consts[:, C_H2PAD:C_H2PAD + OUT_CH]

            idx_t = cpool.tile([P, 8 * S], i16)
            nc.sync.dma_start(out=idx_t[:], in_=idxP[:, :])
            par_t = cpool.tile([P, 2 * S], bf16)
            nc.sync.dma_start(out=par_t[:], in_=parP[:, :])

            ident_b = cpool.tile([P, P], bf16)
            make_identity(nc, ident_b[:])
            if DEBUG_PHASE >= 2:
                t2stage = cpool.tile([P, K * OUT_CH], bf16)
                adst2O = cpool.tile([P, K], f32)
            adstO = cpool.tile([P, HEADS * K], f32)

            # ---------------- phase A: build T1 rows (h) for all positions --
            NSPLIT = max(1, min(8, n_tiles // 8))
            bounds = [n_tiles * i // NSPLIT for i in range(NSPLIT + 1)]
            with tc.tile_pool(name="pa_x", bufs=3) as xpool, \
                 tc.tile_pool(name="pa_st", bufs=2) as stpool, \
                 tc.tile_pool(name="pa_ps", bufs=2, space="PSUM") as pspool:
                GRP = 8
                for s_ in range(NSPLIT):
                    lo, hi = bounds[s_], bounds[s_ + 1]
                    stg = stpool.tile([P, (hi - lo) * HM], bf16, tag="stg")
                    for t0 in range(lo, hi, GRP):
                        g = min(GRP, hi - t0)
                        xt = xpool.tile([IN_CH, g * P], f32, tag="xt")
                        nc.sync.dma_start(out=xt[:],
                                          in_=xT[:, t0 * P:(t0 + g) * P])
                        for j in range(g):
                            ps = pspool.tile([P, HM], f32, tag="ps")
                            nc.tensor.matmul(ps[:],
                                             lhsT=xt[:, j * P:(j + 1) * P],
                                             rhs=w1_t, start=True, stop=True)
                            t = t0 + j
                            nc.scalar.copy(
                                stg[:, (t - lo) * HM:(t - lo + 1) * HM], ps[:])
                            # pad rows: h := h_pad so gathered pad slots get
                            # a_src == -PADM
                            if (t % K) == K - 1 and n_pad:
                                sl = stg[:, (t - lo) * HM:(t - lo + 1) * HM]
                                nc.vector.tensor_scalar_mul(
                                    sl, sl, m01_t[:, 0:1])
                                nc.vector.tensor_add(sl, sl, hpad_t)
                    dview = T1[lo * P:hi * P, :].rearrange(
                        "(t p) f -> p t f", p=P)
                    nc.sync.dma_start(
                        out=dview,
                        in_=stg[:].rearrange("p (t f) -> p t f", f=HM))

                # phase A2: a_dst of owned nodes, per chunk
                xto = xpool.tile([IN_CH, ppc], f32, tag="xto")
                nc.sync.dma_start(out=xto[:], in_=xTo[:, :])
                for k in range(K):
                    ps2 = pspool.tile([P, HEADS], f32, tag="ps2")
                    nc.tensor.matmul(ps2[:],
                                     lhsT=xto[:, k * P:(k + 1) * P],
                                     rhs=a1d_t, start=True, stop=True)
                    nc.vector.tensor_copy(
                        adstO[:, HEADS * k:HEADS * (k + 1)], ps2[:])

            # ---------------- phase B: layer-1 aggregation ------------------
            with tc.tile_pool(name="pb_g", bufs=2) as gpool, \
                 tc.tile_pool(name="pb_sm", bufs=3) as smpool, \
                 tc.tile_pool(name="pb_ps", bufs=2, space="PSUM") as psB, \
                 tc.tile_pool(name="pb_pst", bufs=2, space="PSUM") as psT, \
                 tc.tile_pool(name="pb_psu", bufs=2, space="PSUM") as psU:
                qrot = 0
                for k in range(K):
                    D = D_list[k]
                    co = offs[k]
                    G = gpool.tile([P, D, 2 * HM], bf16, tag="G")
                    if DEBUG_PHASE < 1:
                        nc.vector.memset(G[:], 0)
                    for d0 in range(0, D, SEG) if DEBUG_PHASE >= 1 else []:
                        d1 = min(d0 + SEG, D)
                        nidx = 128 * (d1 - d0)
                        nc.gpsimd.dma_gather(
                            G[:, d0:d1, :], T1p,
                            idx_t[:, 8 * (co + d0):8 * (co + d1)],
                            nidx, nidx, 2 * HM, queue_num=qrot % 4)
                        qrot += 1
                    if DEBUG_PHASE < 2:
                        dbg = smpool.tile([P, OUT_CH], f32, tag="dbg")
                        nc.vector.tensor_copy(dbg[:], G[:, 0, 0:OUT_CH])
                        nc.sync.dma_start(out=outP[k * P:(k + 1) * P, :],
                                          in_=dbg[:])
                        continue
                    ev = G[:, :, 0:HM]
                    od = G[:, :, HM:2 * HM]
                    mi_b = par_t[:, S + co:S + co + D].unsqueeze(2) \
                        .to_broadcast([P, D, HM])
                    m_b = par_t[:, co:co + D].unsqueeze(2) \
                        .to_broadcast([P, D, HM])
                    nc.vector.tensor_tensor(out=ev, in0=ev, in1=mi_b,
                                            op=Alu.mult)
                    nc.vector.tensor_tensor(out=od, in0=od, in1=m_b,
                                            op=Alu.mult)
                    nc.vector.tensor_tensor(out=ev, in0=ev, in1=od,
                                            op=Alu.add)

                    # logits: a_src via mult+reduce, + a_dst, leaky-relu
                    logits = smpool.tile([P, HEADS * D], f32, tag="logits")
                    tmpm = smpool.tile([P, D, MID], f32, tag="tmpm")
                    for h in range(HEADS):
                        a_b = a1s_t[:, h * MID:(h + 1) * MID].unsqueeze(1) \
                            .to_broadcast([P, D, MID])
                        nc.vector.tensor_tensor(
                            out=tmpm[:], in0=ev[:, :, h * MID:(h + 1) * MID],
                            in1=a_b, op=Alu.mult)
                        lh = logits[:, h * D:(h + 1) * D]
                        nc.vector.tensor_reduce(
                            lh, tmpm[:], axis=mybir.AxisListType.X,
                            op=Alu.add)
                        nc.vector.tensor_scalar_add(
                            lh, lh, adstO[:, HEADS * k + h:HEADS * k + h + 1])
                        nc.vector.scalar_tensor_tensor(
                            lh, lh, NEG_SLOPE, lh, op0=Alu.mult, op1=Alu.max)
                    negmax = smpool.tile([P, HEADS], f32, tag="negmax")
                    for h in range(HEADS):
                        nc.vector.reduce_max(
                            negmax[:, h:h + 1], logits[:, h * D:(h + 1) * D],
                            axis=mybir.AxisListType.X, negate=True)
                    e_t = smpool.tile([P, HEADS * D], f32, tag="e")
                    s_t = smpool.tile([P, HEADS], f32, tag="s")
                    for h in range(HEADS):
                        nc.scalar.activation(
                            e_t[:, h * D:(h + 1) * D],
                            logits[:, h * D:(h + 1) * D],
                            Act.Exp, bias=negmax[:, h:h + 1],
                            accum_out=s_t[:, h:h + 1])
                    rcp = smpool.tile([P, HEADS], f32, tag="rcp")
                    nc.vector.reciprocal(rcp[:], s_t[:])

                    # scale gathered h in place by unnormalized attention
                    hview = ev.rearrange("p d (h c) -> p d h c", c=MID)
                    e_b = e_t[:].rearrange("p (h d) -> p d h", d=D) \
                        .unsqueeze(3).to_broadcast([P, D, HEADS, MID])
                    nc.vector.tensor_tensor(out=hview, in0=hview, in1=e_b,
                                            op=Alu.mult)

                    ps = psB.tile([P, HM], f32, tag="acc")
                    for d in range(D):
                        nc.tensor.matmul(ps[:], lhsT=ident_b[:],
                                         rhs=ev[:, d, :],
                                         start=(d == 0), stop=(d == D - 1))

                    tmp = smpool.tile([P, HM], f32, tag="tmp")
                    rcp_b = rcp[:].unsqueeze(2).to_broadcast([P, HEADS, MID])
                    nc.vector.tensor_tensor(
                        out=tmp[:].rearrange("p (h c) -> p h c", c=MID),
                        in0=ps[:].rearrange("p (h c) -> p h c", c=MID),
                        in1=rcp_b, op=Alu.mult)
                    nc.vector.tensor_add(tmp[:], tmp[:], b1r_t)
                    r1 = smpool.tile([P, HM], bf16, tag="r1")
                    nc.scalar.activation(r1[:], tmp[:], Act.Relu)

                    tps = psT.tile([P, P], bf16, tag="tps")
                    nc.tensor.transpose(tps[:], r1[:], ident_b[:])
                    r1T = smpool.tile([P, P], f32, tag="r1T")
                    nc.vector.tensor_copy(r1T[:], tps[:])
                    t2ps = psU.tile([P, OUT_CH + 1], f32, tag="t2ps")
                    nc.tensor.matmul(t2ps[:], lhsT=r1T[:], rhs=w2a_t,
                                     start=True, stop=True)
                    nc.scalar.copy(
                        t2stage[:, k * OUT_CH:(k + 1) * OUT_CH],
                        t2ps[:, 0:OUT_CH])
                    nc.vector.tensor_copy(adst2O[:, k:k + 1],
                                          t2ps[:, OUT_CH:OUT_CH + 1])

            if DEBUG_PHASE >= 2:
                if n_pad:
                    # pad rows: h2 := h2_pad so gathered pad slots get
                    # a_src2 == -PADM
                    sl2 = t2stage[:, (K - 1) * OUT_CH:K * OUT_CH]
                    nc.vector.tensor_scalar_mul(sl2, sl2, m01_t[:, 0:1])
                    nc.vector.tensor_add(sl2, sl2, h2pad_t)
                nc.sync.dma_start(
                    out=T2s[:, :].rearrange("(k p) f -> p k f", p=P),
                    in_=t2stage[:].rearrange("p (k f) -> p k f", f=OUT_CH))

                nc.gpsimd.collective_compute(
                    "AllGather",
                    mybir.AluOpType.bypass,
                    replica_groups=[list(range(N_CORES))],
                    ins=[T2s[:, :]],
                    outs=[T2[:, :]],
                )

            # ---------------- phase C: layer-2 aggregation ------------------
            if DEBUG_PHASE >= 3:
              with tc.tile_pool(name="pc_g", bufs=2) as g2pool, \
                 tc.tile_pool(name="pc_sm", bufs=3) as sm2pool, \
                 tc.tile_pool(name="pc_ps", bufs=2, space="PSUM") as psC:
                qrot = 0
                for k in range(K):
                    D = D_list[k]
                    co = offs[k]
                    G2 = g2pool.tile([P, D, 2 * OUT_CH], bf16, tag="G2")
                    for d0 in range(0, D, SEG):
                        d1 = min(d0 + SEG, D)
                        nidx = 128 * (d1 - d0)
                        nc.gpsimd.dma_gather(
                            G2[:, d0:d1, :], T2p,
                            idx_t[:, 8 * (co + d0):8 * (co + d1)],
                            nidx, nidx, 2 * OUT_CH, queue_num=qrot % 4)
                        qrot += 1
                    ev2 = G2[:, :, 0:OUT_CH]
                    od2 = G2[:, :, OUT_CH:2 * OUT_CH]
                    mi_b = par_t[:, S + co:S + co + D].unsqueeze(2) \
                        .to_broadcast([P, D, OUT_CH])
                    m_b = par_t[:, co:co + D].unsqueeze(2) \
                        .to_broadcast([P, D, OUT_CH])
                    nc.vector.tensor_tensor(out=ev2, in0=ev2, in1=mi_b,
                                            op=Alu.mult)
                    nc.vector.tensor_tensor(out=od2, in0=od2, in1=m_b,
                                            op=Alu.mult)
                    nc.vector.tensor_tensor(out=ev2, in0=ev2, in1=od2,
                                            op=Alu.add)

                    logits2 = sm2pool.tile([P, D], f32, tag="logits2")
                    tmp2 = sm2pool.tile([P, D, OUT_CH], f32, tag="tmp2")
                    a2_b = a2s_t[:, :].unsqueeze(1) \
                        .to_broadcast([P, D, OUT_CH])
                    nc.vector.tensor_tensor(out=tmp2[:], in0=ev2, in1=a2_b,
                                            op=Alu.mult)
                    nc.vector.tensor_reduce(
                        logits2[:], tmp2[:], axis=mybir.AxisListType.X,
                        op=Alu.add)
                    nc.vector.tensor_scalar_add(logits2[:], logits2[:],
                                                adst2O[:, k:k + 1])
                    nc.vector.scalar_tensor_tensor(
                        logits2[:], logits2[:], NEG_SLOPE, logits2[:],
                        op0=Alu.mult, op1=Alu.max)
                    negmax2 = sm2pool.tile([P, 1], f32, tag="negmax2")
                    nc.vector.reduce_max(negmax2[:], logits2[:],
                                         axis=mybir.AxisListType.X,
                                         negate=True)
                    e2 = sm2pool.tile([P, D], f32, tag="e2")
                    s2 = sm2pool.tile([P, 1], f32, tag="s2")
                    nc.scalar.activation(e2[:], logits2[:], Act.Exp,
                                         bias=negmax2[:, 0:1],
                                         accum_out=s2[:, 0:1])
                    rcp2 = sm2pool.tile([P, 1], f32, tag="rcp2")
                    nc.vector.reciprocal(rcp2[:], s2[:])

                    e2_b = e2[:].unsqueeze(2).to_broadcast([P, D, OUT_CH])
                    nc.vector.tensor_tensor(out=ev2, in0=ev2, in1=e2_b,
                                            op=Alu.mult)

                    ps2 = psC.tile([P, OUT_CH], f32, tag="acc2")
                    for d in range(D):
                        nc.tensor.matmul(ps2[:], lhsT=ident_b[:],
                                         rhs=ev2[:, d, :],
                                         start=(d == 0), stop=(d == D - 1))

                    outt = sm2pool.tile([P, OUT_CH], f32, tag="outt")
                    nc.scalar.activation(outt[:], ps2[:], Act.Identity,
                                         scale=rcp2[:, 0:1])
                    nc.vector.tensor_add(outt[:], outt[:], b2r_t)
                    nc.sync.dma_start(out=outP[k * P:(k + 1) * P, :],
                                      in_=outt[:])

    return _finish(nc)


def _make_runner(nc, n_cores):
    import jax
    from jax.sharding import Mesh, PartitionSpec
    from jax.experimental.shard_map import shard_map
    from concourse import bass2jax
    import concourse.mybir as mybir

    bass2jax.install_neuronx_cc_hook()
    partition_name = (nc.partition_id_tensor.name
                      if nc.partition_id_tensor else None)
    in_names = []
    out_names = []
    out_avals = []
    zero_outs = []
    for alloc in nc.m.functions[0].allocations:
        if not isinstance(alloc, mybir.MemoryLocationSet):
            continue
        name = alloc.memorylocations[0].name
        if alloc.kind == "ExternalInput":
            if name != partition_name:
                in_names.append(name)
        elif alloc.kind == "ExternalOutput":
            shape = tuple(alloc.tensor_shape)
            dtype = mybir.dt.np(alloc.dtype)
            out_names.append(name)
            out_avals.append(jax.core.ShapedArray(shape, dtype))
            zero_outs.append(np.zeros(shape, dtype))
    n_params = len(in_names)
    all_names = list(in_names) + out_names
    if partition_name is not None:
        all_names.append(partition_name)

    def _body(*args):
        operands = list(args)
        if partition_name is not None:
            operands.append(bass2jax.partition_id_tensor())
        outs = bass2jax._bass_exec_p.bind(
            *operands,
            out_avals=tuple(out_avals),
            in_names=tuple(all_names),
            out_names=tuple(out_names),
            lowering_input_output_aliases=(),
            sim_require_finite=True,
            sim_require_nnan=True,
            nc=nc,
        )
        return tuple(outs)

    devices = jax.devices()[:n_cores]
    mesh = Mesh(np.asarray(devices), ("core",))
    nio = n_params + len(out_names)
    sharded = jax.jit(
        shard_map(_body, mesh=mesh, in_specs=(PartitionSpec("core"),) * nio,
                  out_specs=(PartitionSpec("core"),) * len(out_names),
                  check_rep=False),
        keep_unused=True,
    )
    return dict(fn=sharded, in_names=in_names, out_names=out_names,
                zero_outs=zero_outs, mesh=mesh, n_cores=n_cores)


def _execute(runner, in_maps):
    n_cores = runner["n_cores"]
    concat_in = [
        np.concatenate([np.asarray(in_maps[c][name])
                        for c in range(n_cores)], axis=0)
        for name in runner["in_names"]
    ]
    concat_zeros = [
        np.zeros((n_cores * z.shape[0], *z.shape[1:]), z.dtype)
        for z in runner["zero_outs"]
    ]
    out_arrs = runner["fn"](*concat_in, *concat_zeros)
    out_arrs = [np.asarray(a) for a in out_arrs]
    res = []
    for c in range(n_cores):
        m = {}
        for i, name in enumerate(runner["out_names"]):
            a = out_arrs[i]
            s0 = a.shape[0] // n_cores
            m[name] = a[c * s0:(c + 1) * s0]
        res.append(m)
    return res


def _time_exec(runner, in_maps, iters=5):
    """Steady-state wall-clock of the compiled NEFF execution (device-resident
    inputs, no host transfers in the loop)."""
    import time as _time

    import jax
    from jax.sharding import NamedSharding, PartitionSpec

    n_cores = runner["n_cores"]
    sh = NamedSharding(runner["mesh"], PartitionSpec("core"))
    concat_in = [
        np.concatenate([np.asarray(in_maps[c][name])
                        for c in range(n_cores)], axis=0)
        for name in runner["in_names"]
    ]
    concat_zeros = [
        np.zeros((n_cores * z.shape[0], *z.shape[1:]), z.dtype)
        for z in runner["zero_outs"]
    ]
    dev_in = [jax.device_put(a, sh) for a in concat_in]
    dev_z = [jax.device_put(a, sh) for a in concat_zeros]
    times = []
    for _ in range(iters):
        t0 = _time.perf_counter()
        outs = runner["fn"](*dev_in, *dev_z)
        for o in outs:
            o.block_until_ready()
        times.append(_time.perf_counter() - t0)
    return min(times), times


def _get_compiled(inputs):
    x = np.asarray(inputs["x"], np.float32)
    prep = _host_prep(x, np.asarray(inputs["edge_index"]))
    key = (prep["K"], prep["D_list"], prep["n_pos"], prep["ppc"],
           prep["n_own"], prep["n_pad"])
    if key not in _cache:
        nc = _build_program(prep["K"], prep["D_list"], prep["offs"],
                            prep["S"], prep["n_pos"], prep["ppc"],
                            prep["n_own"], prep["n_pad"])
        _cache[key] = _make_runner(nc, N_CORES)
    runner = _cache[key]

    import ml_dtypes

    W1 = np.asarray(inputs["W1"], np.float32)
    W2 = np.asarray(inputs["W2"], np.float32)
    a_src1 = np.asarray(inputs["a_src1"], np.float32)
    a_dst1 = np.asarray(inputs["a_dst1"], np.float32)
    a_src2 = np.asarray(inputs["a_src2"], np.float32).reshape(-1)
    a_dst2 = np.asarray(inputs["a_dst2"], np.float32).reshape(-1)
    HM = HEADS * MID

    A1s = _feat_mat(a_src1, HEADS, MID)          # [128, 4]
    A1d = _feat_mat(a_dst1, HEADS, MID)
    # pad rows: h with A1s^T h = -PADM per head (least-norm);
    # layer-2 pad rows: h2 with a2s^T h2 = -PADM
    h_pad = (A1s @ np.linalg.solve(A1s.T @ A1s,
                                   np.full(HEADS, -PADM, np.float64))
             ).astype(np.float32)
    h2_pad = ((-PADM / (a_src2 @ a_src2)) * a_src2).astype(np.float32)

    n_own, ppc, n_pad = prep["n_own"], prep["ppc"], prep["n_pad"]
    xT = prep["xT"]

    C_END = (HM + HEADS + (OUT_CH + 1) + HM + OUT_CH + HM + OUT_CH
             + 1 + HM + OUT_CH)
    consts = np.zeros((P, C_END), np.float32)
    o = 0
    consts[:, o:o + HM] = W1; o += HM
    consts[:, o:o + HEADS] = W1 @ A1d; o += HEADS
    consts[:, o:o + OUT_CH] = W2
    consts[:, o + OUT_CH:o + OUT_CH + 1] = (W2 @ a_dst2)[:, None]
    o += OUT_CH + 1
    consts[:, o:o + HM] = np.broadcast_to(a_src1.reshape(-1)[None, :],
                                          (P, HM)); o += HM
    consts[:, o:o + OUT_CH] = np.broadcast_to(a_src2[None, :], (P, OUT_CH))
    o += OUT_CH
    consts[:, o:o + HM] = np.broadcast_to(
        np.asarray(inputs["b1"], np.float32)[None, :], (P, HM)); o += HM
    consts[:, o:o + OUT_CH] = np.broadcast_to(
        np.asarray(inputs["b2"], np.float32)[None, :], (P, OUT_CH))
    o += OUT_CH
    r = n_own % P
    m01 = np.ones(P, np.float32)
    padt1 = np.zeros((P, HM), np.float32)
    padt2 = np.zeros((P, OUT_CH), np.float32)
    if n_pad:
        m01[r:] = 0.0
        padt1[r:, :] = h_pad[None, :]
        padt2[r:, :] = h2_pad[None, :]
    consts[:, o:o + 1] = m01[:, None]; o += 1
    consts[:, o:o + HM] = padt1; o += HM
    consts[:, o:o + OUT_CH] = padt2; o += OUT_CH

    S = prep["S"]
    in_maps = []
    for c in range(N_CORES):
        pm = np.empty((P, 2 * S), np.float32)
        pm[:, 0:S] = prep["par"][c]
        pm[:, S:2 * S] = 1.0 - prep["par"][c]
        in_maps.append({
            "xT": xT,
            "xTown": np.ascontiguousarray(xT[:, c * ppc:(c + 1) * ppc]),
            "consts": consts,
            "idxw": prep["idxw"][c],
            "pmask": pm.astype(ml_dtypes.bfloat16),
        })
    return runner, in_maps, prep


def _run(inputs):
    runner, in_maps, prep = _get_compiled(inputs)
    import time as _time
    last_exc = None
    for attempt in range(3):
        try:
            results = _execute(runner, in_maps)
            break
        except Exception as exc:
            last_exc = exc
            _time.sleep(2.0 + 4.0 * attempt)
    else:
        raise last_exc
    out = np.empty((N, OUT_CH), np.float32)
    n_own, ppc = prep["n_own"], prep["ppc"]
    for c in range(N_CORES):
        o = np.asarray(results[c]["out"])
        nodes = prep["node_at_pos"][c * ppc:c * ppc + n_own]
        out[nodes] = o[:n_own]
    return out


def kernel(**inputs):
    return _run(inputs)
